# revision 1
# baseline (speedup 1.0000x reference)
"""Trainium2 Bass kernel for nn_HadaMard: fused proj + 2xLayerNorm + outer product.

Reference computation (per batch b):
  qf = q[b].reshape(C1, N)           # [1024, 1024]  (C1 on rows, N=H*W cols)
  proj = Wp @ qf + bp                # [256, 1024]
  qn = LN_over_d(proj) * g1 + b1     # LN over the 256-channel dim
  xn = LN_over_e(x[b]) * g2 + b2     # LN over the 32-channel dim
  out[d*32+e, n] = qn[d, n] * xn[e, n]   # [8192, 1024]

Sharding: data-parallel over B=8, one batch per NeuronCore.

On-chip layout is [channel, n] everywhere (zero transposes):
  - proj = WpT.T @ q via PE (WpT host-transposed, q natural layout)
  - LN stats over the partition axis via ones-matmuls: lhsT = ones*(1/C)
    gives the mean broadcast to all 128 partitions for free.
  - outer product: stationary S4 [4,128] (S4[j,p] = 1 if p//32 == j) broadcasts
    4 qn rows -> 128 partitions in PSUM; one DVE tensor_mul against a
    replicated xn tile -> output tile [128, 1024] -> contiguous 512KB DMA.
"""

import numpy as np

_CACHE = {}

B, C1, H, W = 8, 1024, 32, 32
C2 = 32
Cp = 256
N = H * W  # 1024
CD = Cp * C2  # 8192
EPS = 1e-5


def _build_nc(trace_label=False):
    import os

    import concourse.bacc as bacc
    import concourse.bass as bass
    import concourse.mybir as mybir
    import concourse.tile as tile

    f32r_proj = os.environ.get("HM_F32R_PROJ", "0") == "1"
    f32r_stats = os.environ.get("HM_F32R_STATS", "0") == "1"
    f32r_sel = os.environ.get("HM_F32R_SEL", "0") == "1"
    simple = os.environ.get("HM_SIMPLE", "0") == "1"  # g1=1,b1=0,g2=1,b2=0,bp=0
    split = os.environ.get("HM_SPLIT", "1") == "1"  # bf16 hi/lo selection matmuls

    F32 = mybir.dt.float32
    F32R = mybir.dt.float32r
    BF16 = mybir.dt.bfloat16
    MULT = mybir.AluOpType.mult
    ADD = mybir.AluOpType.add
    SQRT = mybir.ActivationFunctionType.Sqrt

    nc = bacc.Bacc(None, target_bir_lowering=False)

    qh_d = nc.dram_tensor("qh", [C1, N], BF16, kind="ExternalInput")
    ql_d = nc.dram_tensor("ql", [C1, N], BF16, kind="ExternalInput")
    x_d = nc.dram_tensor("x", [C2, N], F32, kind="ExternalInput")
    wh_d = nc.dram_tensor("wh", [C1, Cp], BF16, kind="ExternalInput")
    wl_d = nc.dram_tensor("wl", [C1, Cp], BF16, kind="ExternalInput")
    bp_d = nc.dram_tensor("bpc", [128, 2], F32, kind="ExternalInput")
    g1_d = nc.dram_tensor("g1c", [128, 2], F32, kind="ExternalInput")
    b1_d = nc.dram_tensor("b1c", [128, 2], F32, kind="ExternalInput")
    g2_d = nc.dram_tensor("g2r", [128, 1], F32, kind="ExternalInput")
    b2_d = nc.dram_tensor("b2r", [128, 1], F32, kind="ExternalInput")
    rep_d = nc.dram_tensor(
        "rep", [128, 16 * 128], BF16 if split else F32, kind="ExternalInput"
    )
    sx_d = nc.dram_tensor("sx", [C2, 128], F32, kind="ExternalInput")
    out_d = nc.dram_tensor("out", [CD, N], F32, kind="ExternalOutput")

    with tile.TileContext(nc) as tc:
        with (
            tc.tile_pool(name="cst", bufs=1) as cst,
            tc.tile_pool(name="big", bufs=1) as big,
            tc.tile_pool(name="wrk", bufs=2) as wrk,
            tc.tile_pool(name="stt", bufs=1) as stt,
            tc.tile_pool(name="ost", bufs=4) as ost,
            tc.tile_pool(name="ps", bufs=4, space=bass.MemorySpace.PSUM) as ps,
        ):
            # ---- input loads ----
            qh_sb, ql_sb, wh_sb, wl_sb = [], [], [], []
            for k in range(8):
                t = big.tile([128, N], BF16, tag=f"qh{k}")
                nc.sync.dma_start(t[:], qh_d[128 * k : 128 * (k + 1), :])
                qh_sb.append(t)
                t = big.tile([128, N], BF16, tag=f"ql{k}")
                nc.scalar.dma_start(t[:], ql_d[128 * k : 128 * (k + 1), :])
                ql_sb.append(t)
                t = big.tile([128, Cp], BF16, tag=f"wh{k}")
                nc.sync.dma_start(t[:], wh_d[128 * k : 128 * (k + 1), :])
                wh_sb.append(t)
                t = big.tile([128, Cp], BF16, tag=f"wl{k}")
                nc.scalar.dma_start(t[:], wl_d[128 * k : 128 * (k + 1), :])
                wl_sb.append(t)
            x_sb = cst.tile([C2, N], F32, tag="xs")
            nc.sync.dma_start(x_sb[:], x_d[:])

            def cload(dram, shape, tag):
                t = cst.tile(shape, F32, tag=tag)
                nc.sync.dma_start(t[:], dram[:])
                return t

            bp_sb = cload(bp_d, [128, 2], "bp")
            g1_sb = cload(g1_d, [128, 2], "g1")
            b1_sb = cload(b1_d, [128, 2], "b1")
            g2_sb = cload(g2_d, [128, 1], "g2")
            b2_sb = cload(b2_d, [128, 1], "b2")
            cq_sb = cst.tile([128, 128], F32, tag="cq")
            nc.vector.memset(cq_sb[:], 1.0 / Cp)
            cx_sb = cst.tile([C2, 128], F32, tag="cx")
            nc.vector.memset(cx_sb[:], 1.0 / C2)
            rep_sb = cst.tile([128, 16 * 128], BF16 if split else F32, tag="rep")
            nc.sync.dma_start(rep_sb[:], rep_d[:])
            sx_sb = cload(sx_d, [C2, 128], "sx")
            eps_t = cst.tile([128, 1], F32, tag="eps")
            nc.vector.memset(eps_t[:], EPS)

            def mm_dt(ap, on):
                return ap.bitcast(F32R) if on else ap

            # ---- projection: proj[d, n] = sum_c WpT[c, d] * q[c, n]  (+bp) ----
            projb = []
            for md in range(2):
                pj = ps.tile([128, N], F32, tag="ps")
                for k in range(8):
                    lh = wh_sb[k][:, 128 * md : 128 * (md + 1)]
                    ll = wl_sb[k][:, 128 * md : 128 * (md + 1)]
                    for h in range(2):
                        hs = slice(512 * h, 512 * (h + 1))
                        # wh@qh + wh@ql + wl@qh (ll term ~2^-16, dropped)
                        nc.tensor.matmul(pj[:, hs], lh, qh_sb[k][:, hs],
                                         start=(k == 0), stop=False)
                        nc.tensor.matmul(pj[:, hs], lh, ql_sb[k][:, hs],
                                         start=False, stop=False)
                        nc.tensor.matmul(pj[:, hs], ll, qh_sb[k][:, hs],
                                         start=False, stop=(k == 7))
                pb = stt.tile([128, N], F32, tag=f"pb{md}")
                if simple:
                    nc.vector.tensor_copy(pb[:], pj[:])
                else:
                    nc.vector.tensor_scalar_add(pb[:], pj[:], bp_sb[:, md : md + 1])
                projb.append(pb)

            # squares (ScalarE, keeps DVE free)
            sq = []
            for md in range(2):
                s = wrk.tile([128, N], F32, tag=f"sq{md}")
                nc.scalar.square(s[:], projb[md][:])
                sq.append(s)

            # stats via ones-matmuls: mean & E[v^2], broadcast to 128 partitions
            smq = ps.tile([128, N], F32, tag="ps")
            for md in range(2):
                for h in range(2):
                    nc.tensor.matmul(
                        smq[:, 512 * h : 512 * (h + 1)],
                        mm_dt(cq_sb[:], f32r_stats),
                        mm_dt(projb[md][:, 512 * h : 512 * (h + 1)], f32r_stats),
                        start=(md == 0),
                        stop=(md == 1),
                    )
            sqq = ps.tile([128, N], F32, tag="ps")
            for md in range(2):
                for h in range(2):
                    nc.tensor.matmul(
                        sqq[:, 512 * h : 512 * (h + 1)],
                        mm_dt(cq_sb[:], f32r_stats),
                        mm_dt(sq[md][:, 512 * h : 512 * (h + 1)], f32r_stats),
                        start=(md == 0),
                        stop=(md == 1),
                    )

            mb = stt.tile([128, N], F32, tag="mb")
            nc.vector.tensor_copy(mb[:], smq[:])
            m2 = wrk.tile([128, N], F32, tag="t")
            nc.scalar.square(m2[:], mb[:])
            var = wrk.tile([128, N], F32, tag="t2")
            nc.vector.tensor_sub(var[:], sqq[:], m2[:])
            sd = wrk.tile([128, N], F32, tag="t")
            nc.scalar.activation(sd[:], var[:], SQRT, bias=eps_t[:])
            rstd = stt.tile([128, N], F32, tag="rstd")
            rscr = wrk.tile([128, N], F32, tag="t3")
            nc.vector.reciprocal_approx_accurate(rstd[:], sd[:], rscr[:])

            # simple mode: qn holds (projb - mean); rstd is folded into XR so the
            # per-tile multiply produces (projb-m)*rstd*xn in one op.
            qn = []
            qn_lo = []
            for md in range(2):
                qq = stt.tile([128, N], F32, tag=f"qn{md}")
                nc.vector.tensor_sub(qq[:], projb[md][:], mb[:])
                if not simple:
                    nc.vector.tensor_mul(qq[:], qq[:], rstd[:])
                    nc.vector.tensor_scalar(
                        qq[:], qq[:], g1_sb[:, md : md + 1], b1_sb[:, md : md + 1],
                        op0=MULT, op1=ADD,
                    )
                if split:
                    # bf16 hi/lo decomposition: qq = hi + lo, |lo| <~ 2^-8 |qq|
                    qh = stt.tile([128, N], BF16, tag=f"qh{md}")
                    nc.vector.tensor_copy(qh[:], qq[:])
                    ql = stt.tile([128, N], BF16, tag=f"ql{md}")
                    nc.vector.tensor_sub(ql[:], qq[:], qh[:])
                    qn.append(qh)
                    qn_lo.append(ql)
                else:
                    qn.append(qq)

            # ---- x LayerNorm (over 32 channels) + partition replication ----
            xsq = wrk.tile([C2, N], F32, tag="xq")
            nc.scalar.square(xsq[:], x_sb[:])
            smx = ps.tile([128, N], F32, tag="ps")
            for h in range(2):
                nc.tensor.matmul(
                    smx[:, 512 * h : 512 * (h + 1)], mm_dt(cx_sb[:], f32r_stats),
                    mm_dt(x_sb[:, 512 * h : 512 * (h + 1)], f32r_stats),
                    start=True, stop=True,
                )
            sqx = ps.tile([128, N], F32, tag="ps")
            for h in range(2):
                nc.tensor.matmul(
                    sqx[:, 512 * h : 512 * (h + 1)], mm_dt(cx_sb[:], f32r_stats),
                    mm_dt(xsq[:, 512 * h : 512 * (h + 1)], f32r_stats),
                    start=True, stop=True,
                )
            xb = ps.tile([128, N], F32, tag="ps")
            for h in range(2):
                nc.tensor.matmul(
                    xb[:, 512 * h : 512 * (h + 1)],
                    mm_dt(sx_sb[:], f32r_sel),
                    mm_dt(x_sb[:, 512 * h : 512 * (h + 1)], f32r_sel),
                    start=True, stop=True,
                )

            mxb = wrk.tile([128, N], F32, tag="mx")
            nc.vector.tensor_copy(mxb[:], smx[:])
            mx2 = wrk.tile([128, N], F32, tag="t")
            nc.scalar.square(mx2[:], mxb[:])
            vx = wrk.tile([128, N], F32, tag="t2")
            nc.vector.tensor_sub(vx[:], sqx[:], mx2[:])
            sdx = wrk.tile([128, N], F32, tag="t")
            nc.scalar.activation(sdx[:], vx[:], SQRT, bias=eps_t[:])
            rsx = wrk.tile([128, N], F32, tag="t3")
            rscx = wrk.tile([128, N], F32, tag="t4")
            nc.vector.reciprocal_approx_accurate(rsx[:], sdx[:], rscx[:])
            xt = wrk.tile([128, N], F32, tag="t2")
            nc.vector.tensor_sub(xt[:], xb[:], mxb[:])
            xnr = stt.tile([128, N], F32, tag="xnr")
            nc.vector.tensor_mul(xnr[:], xt[:], rsx[:])
            if simple:
                # fold q-side rstd into the shared multiplier tile
                nc.vector.tensor_mul(xnr[:], xnr[:], rstd[:])
            else:
                nc.vector.tensor_scalar(
                    xnr[:], xnr[:], g2_sb[:, 0:1], b2_sb[:, 0:1], op0=MULT, op1=ADD
                )

            # ---- outer product: 64 output tiles of [128, 1024] ----
            # tile t = (md, g, r): output rows 128t..128(t+1), qn rows
            # 128md + 32g + 4r + {0..3}. lhsT and rhs share base partition 32g
            # (tile_position constraint); rep_sb holds the selection matrices
            # replicated vertically 4x so any 32-row slice works.
            out_dma_engines = [nc.sync, nc.scalar]
            ot = None
            for md in range(2):
                for g in range(2):
                    for r in range(16):
                        qb = ps.tile([128, N], F32, tag="ps")
                        lhsT = rep_sb[64 * g : 64 * (g + 1), 128 * r : 128 * (r + 1)]
                        for h in range(2):
                            if split:
                                nc.tensor.matmul(
                                    qb[:, 512 * h : 512 * (h + 1)],
                                    lhsT,
                                    qn[md][64 * g : 64 * (g + 1), 512 * h : 512 * (h + 1)],
                                    start=True,
                                    stop=False,
                                )
                                nc.tensor.matmul(
                                    qb[:, 512 * h : 512 * (h + 1)],
                                    lhsT,
                                    qn_lo[md][64 * g : 64 * (g + 1), 512 * h : 512 * (h + 1)],
                                    start=False,
                                    stop=True,
                                )
                            else:
                                nc.tensor.matmul(
                                    qb[:, 512 * h : 512 * (h + 1)],
                                    mm_dt(lhsT, f32r_sel),
                                    mm_dt(qn[md][64 * g : 64 * (g + 1), 512 * h : 512 * (h + 1)], f32r_sel),
                                    start=True,
                                    stop=True,
                                )
                        t = md * 32 + g * 16 + r
                        if t % 2 == 0:
                            ot = ost.tile([128, 2 * N], F32)
                        nc.vector.tensor_mul(
                            ot[:, (t % 2) * N : (t % 2 + 1) * N], qb[:], xnr[:]
                        )
                        if t % 2 == 1:
                            eng = out_dma_engines[(t // 2) % 2]
                            # DRAM rows 128(t-1)+p (half 0) and 128t+p (half 1)
                            # must match SBUF partition p's two 1024-col halves.
                            dst = out_d[128 * (t - 1) : 128 * (t + 1), :].rearrange(
                                "(h p) n -> p h n", h=2
                            )
                            src = ot[:].rearrange("p (h n) -> p h n", h=2)
                            eng.dma_start(dst, src)

    nc.compile()
    return nc


def _host_inputs(q, x, Wp, bp, g1, b1, g2, b2):
    """Build the 8 per-core input maps."""
    import os

    import ml_dtypes
    qf = np.ascontiguousarray(np.asarray(q, dtype=np.float32).reshape(B, C1, N))
    qfh = qf.astype(ml_dtypes.bfloat16)
    qfl = (qf - qfh.astype(np.float32)).astype(ml_dtypes.bfloat16)
    xf = np.ascontiguousarray(np.asarray(x, dtype=np.float32).reshape(B, C2, N))
    wpt = np.ascontiguousarray(np.asarray(Wp, dtype=np.float32).T)
    wh = wpt.astype(ml_dtypes.bfloat16)
    wl = (wpt - wh.astype(np.float32)).astype(ml_dtypes.bfloat16)
    bpc = np.ascontiguousarray(np.asarray(bp, dtype=np.float32).reshape(2, 128).T)
    g1c = np.ascontiguousarray(np.asarray(g1, dtype=np.float32).reshape(2, 128).T)
    b1c = np.ascontiguousarray(np.asarray(b1, dtype=np.float32).reshape(2, 128).T)
    g2r = np.ascontiguousarray(np.tile(np.asarray(g2, dtype=np.float32), 4)[:, None])
    b2r = np.ascontiguousarray(np.tile(np.asarray(b2, dtype=np.float32), 4)[:, None])
    # rep[:, r*128+p]: vertical 2x stack of S64_r, S64_r[k,p] = d(k, 4r + p//32)
    rep = np.zeros((128, 16 * 128), dtype=np.float32)
    for r in range(16):
        for p in range(128):
            k = 4 * r + p // 32
            for v in range(2):
                rep[64 * v + k, 128 * r + p] = 1.0
    if os.environ.get("HM_SPLIT", "1") == "1":
        rep = rep.astype(ml_dtypes.bfloat16)
    sx = np.zeros((C2, 128), dtype=np.float32)
    for p in range(128):
        sx[p % 32, p] = 1.0
    in_maps = []
    for b in range(B):
        in_maps.append(
            {
                "qh": np.ascontiguousarray(qfh[b]),
                "ql": np.ascontiguousarray(qfl[b]),
                "x": xf[b],
                "wh": wh,
                "wl": wl,
                "bpc": bpc,
                "g1c": g1c,
                "b1c": b1c,
                "g2r": g2r,
                "b2r": b2r,
                "rep": rep,
                "sx": sx,
            }
        )
    return in_maps


def _run(in_maps, trace=False):
    import os

    from concourse.bass_utils import run_bass_kernel_spmd

    key = "nc" + os.environ.get("HM_SIMPLE", "0")
    if key not in _CACHE:
        _CACHE[key] = _build_nc()
    nc = _CACHE[key]
    res = run_bass_kernel_spmd(
        nc, in_maps, core_ids=list(range(B)), trace=trace
    )
    return res


def kernel(q, x, Wp, bp, g1, b1, g2, b2):
    import os

    simple = (
        np.allclose(np.asarray(bp), 0)
        and np.allclose(np.asarray(g1), 1)
        and np.allclose(np.asarray(b1), 0)
        and np.allclose(np.asarray(g2), 1)
        and np.allclose(np.asarray(b2), 0)
    )
    os.environ["HM_SIMPLE"] = "1" if simple else "0"
    in_maps = _host_inputs(q, x, Wp, bp, g1, b1, g2, b2)
    res = _run(in_maps, trace=False)
    out = np.stack(
        [res.results[b]["out"].reshape(CD, H, W) for b in range(B)]
    ).astype(np.float32)
    _CACHE["last_res"] = res
    return out



# revision 15
# speedup vs baseline: 2.0258x; 2.0258x over previous
"""Trainium2 Bass kernel for nn_HadaMard: fused proj + 2xLayerNorm + outer product.

Reference computation (per batch b, one NeuronCore per batch):
  qf = q[b].reshape(C1, N)           # [1024, 1024]
  proj = Wp @ qf (+ bp)              # [256, 1024]
  qn = LN_over_d(proj) * g1 + b1     # LN over the 256-channel dim
  xn = LN_over_e(x[b]) * g2 + b2     # LN over the 32-channel dim
  out[d*32+e, n] = qn[d, n] * xn[e, n]   # [8192, 1024]

Layout/strategy:
  - Output is transferred in bf16 (rel-err ~4e-3 << 2e-2 gate) and upcast on
    host: halves the dominant HBM write traffic.
  - Outer-product tiles are e-major: tile (md, e) holds out rows
    (128*md+p)*32+e for p in [0,128). The qn factor is then the qn tile
    itself (no broadcast); the xn factor is one row broadcast to 128
    partitions.
  - simple mode (all-default affine params): rstd(q-LN) is folded into the
    x-side row matrix A'[e,n] = (x-mx)*rsx*rstd, so the q side only needs
    cn = (proj - mean) in bf16.
  - Row broadcasts go through a DRAM scratch roundtrip: A' is written once,
    then each xbe tile is a stride-0 (partition-broadcast) DMA read. These
    land on the DMA queues (sync/scalar/gpsimd) instead of the busy compute
    engines. The first few e's are built via PE selection-matmul + ACT copy
    to hide the roundtrip latency at pipeline start.
  - Elementwise products run on DVE (bf16 2x mode) and Pool, DMAs on
    sync/scalar/gpsimd, assigned by a static least-loaded balancer.
  - n is processed in column chunks to shorten the pipeline head.
"""

import numpy as np

_CACHE = {}

B, C1, H, W = 8, 1024, 32, 32
C2 = 32
Cp = 256
N = H * W  # 1024
CD = Cp * C2  # 8192
EPS = 1e-5

CHUNKS = [(0, 512), (512, 1024)]
PE_ROUTE_E = {0: 4}  # chunk idx -> e's built via PE+ACT instead of DMA
XBE_PREFETCH = 6
NWU = 8  # PE warm-up matmuls


def _build_nc(simple):
    import concourse.bacc as bacc
    import concourse.bass as bass
    import concourse.mybir as mybir
    import concourse.tile as tile

    F32 = mybir.dt.float32
    F32R = mybir.dt.float32r
    BF16 = mybir.dt.bfloat16
    SQRT = mybir.ActivationFunctionType.Sqrt
    COPY = mybir.ActivationFunctionType.Copy
    MULT = mybir.AluOpType.mult
    ADD = mybir.AluOpType.add

    nc = bacc.Bacc(None, target_bir_lowering=False)

    q_d = nc.dram_tensor("qb", [C1, N], BF16, kind="ExternalInput")
    w_d = nc.dram_tensor("wT", [C1, Cp], BF16, kind="ExternalInput")
    x_d = nc.dram_tensor("x", [C2, N], F32, kind="ExternalInput")
    sel_d = nc.dram_tensor("selc", [C2, 4 * 128], BF16, kind="ExternalInput")
    bp_d = nc.dram_tensor("bpc", [128, 2], F32, kind="ExternalInput")
    g1_d = nc.dram_tensor("g1c", [128, 2], F32, kind="ExternalInput")
    b1_d = nc.dram_tensor("b1c", [128, 2], F32, kind="ExternalInput")
    g2_d = nc.dram_tensor("g2r", [C2, 1], F32, kind="ExternalInput")
    b2_d = nc.dram_tensor("b2r", [C2, 1], F32, kind="ExternalInput")
    abuf_d = nc.dram_tensor("abuf", [C2, N], BF16, kind="Internal")
    out_d = nc.dram_tensor("out", [CD, N], BF16, kind="ExternalOutput")

    # out view: row (md*128+p)*32+e  ->  [p, md, e, n]
    outv = out_d.rearrange("(md p e) n -> p md e n", md=2, p=128, e=C2)

    # ---- static least-loaded balancer (model-cost ns) ----
    clk = {"sync": 0.0, "scalar": 0.0, "gpsimd": 0.0, "vector": 0.0}

    def pick(cands, cost):
        e = min(cands, key=lambda x: clk[x])
        clk[e] += cost
        return e

    def charge(e, cost):
        clk[e] += cost

    DMA_ENGS = ["sync", "scalar", "gpsimd"]

    def dma_cost(bytes_per_part):
        return bytes_per_part * 0.3855

    def mul_cost(eng, w):
        if eng == "vector":
            return w * 1.0417 * 0.5 + 60.0
        return w * 0.8333

    with tile.TileContext(nc) as tc:
        with (
            tc.tile_pool(name="cst", bufs=1) as cst,
            tc.tile_pool(name="big", bufs=1) as big,
            tc.tile_pool(name="xbe", bufs=1) as xbp,
            tc.tile_pool(name="stg", bufs=10) as stg,
            tc.tile_pool(name="wrk", bufs=2) as wrk,
            tc.tile_pool(name="ps", bufs=4, space=bass.MemorySpace.PSUM) as ps,
            tc.tile_pool(name="wups", bufs=1, space=bass.MemorySpace.PSUM) as wups,
            tc.tile_pool(name="ps32", bufs=3, space=bass.MemorySpace.PSUM) as ps32,
        ):
            # ---- constants / warmup (t=0, no input deps) ----
            wu_l = cst.tile([128, 128], BF16, tag="wul")
            nc.vector.memset(wu_l[:], 0.5)
            wu_r = cst.tile([128, 512], BF16, tag="wur")
            nc.vector.memset(wu_r[:], 0.5)
            cq128 = cst.tile([128, 128], F32, tag="cq128")
            nc.vector.memset(cq128[:], 1.0 / Cp)
            cq32 = cst.tile([128, C2], F32, tag="cq32")
            nc.vector.memset(cq32[:], 1.0 / Cp)
            cx32 = cst.tile([C2, C2], F32, tag="cx32")
            nc.vector.memset(cx32[:], 1.0 / C2)
            eps32 = cst.tile([C2, 1], F32, tag="eps32")
            nc.vector.memset(eps32[:], EPS)
            eps128 = cst.tile([128, 1], F32, tag="eps128")
            nc.vector.memset(eps128[:], EPS)
            # preload the Sqrt activation table early (ACT, off critical path)
            atl = cst.tile([C2, 1], F32, tag="atl")
            nc.scalar.activation(atl[:], eps32[:], SQRT, bias=eps32[:])

            wu_ps = wups.tile([128, 512], F32, tag="wups")
            for i in range(NWU):
                nc.tensor.matmul(wu_ps[:], wu_l[:], wu_r[:], start=True, stop=True)

            # ---- input loads (balanced across DMA queues) ----
            x_sb = cst.tile([C2, N], F32, tag="xs")
            eng = pick(DMA_ENGS, dma_cost(4096))
            getattr(nc, eng).dma_start(x_sb[:], x_d[:])
            sel_sb = cst.tile([C2, 4 * 128], BF16, tag="sel")
            eng = pick(DMA_ENGS, dma_cost(1024))
            getattr(nc, eng).dma_start(sel_sb[:], sel_d[:])

            w_sb = []
            for k in range(8):
                t = big.tile([128, Cp], BF16, tag=f"w{k}")
                eng = pick(DMA_ENGS, dma_cost(512))
                getattr(nc, eng).dma_start(t[:], w_d[128 * k : 128 * (k + 1), :])
                w_sb.append(t)

            q_sb = []
            for k in range(8):
                qt = big.tile([128, N], BF16, tag=f"q{k}")
                q_sb.append(qt)
            for ci, (c0, c1) in enumerate(CHUNKS):
                for k in range(8):
                    eng = pick(DMA_ENGS, dma_cost((c1 - c0) * 2))
                    getattr(nc, eng).dma_start(
                        q_sb[k][:, c0:c1], q_d[128 * k : 128 * (k + 1), c0:c1]
                    )

            def cload(dram, shape, tag):
                t = cst.tile(shape, F32, tag=tag)
                eng = pick(DMA_ENGS, dma_cost(shape[1] * 4))
                getattr(nc, eng).dma_start(t[:], dram[:])
                return t

            if not simple:
                bp_sb = cload(bp_d, [128, 2], "bp")
                g1_sb = cload(g1_d, [128, 2], "g1")
                b1_sb = cload(b1_d, [128, 2], "b1")
                g2_sb = cload(g2_d, [C2, 1], "g2")
                b2_sb = cload(b2_d, [C2, 1], "b2")

            # ---- x-side LN (independent of q) ----
            xsq = cst.tile([C2, N], F32, tag="xsq")
            nc.gpsimd.tensor_mul(xsq[:], x_sb[:], x_sb[:])
            charge("gpsimd", N * 0.8333)

            xt = cst.tile([C2, N], F32, tag="xt")
            sdx = cst.tile([C2, N], F32, tag="sdx")
            mx = cst.tile([C2, N], F32, tag="mx")
            mx2 = cst.tile([C2, N], F32, tag="mx2")
            vx = cst.tile([C2, N], F32, tag="vx")
            for ci, (c0, c1) in enumerate(CHUNKS):
                w_ = c1 - c0
                smx = ps32.tile([C2, 512], F32, tag="s32")
                nc.tensor.matmul(
                    smx[:, :w_], cx32[:].bitcast(F32R), x_sb[:, c0:c1].bitcast(F32R),
                    start=True, stop=True,
                )
                sqx = ps32.tile([C2, 512], F32, tag="s32")
                nc.tensor.matmul(
                    sqx[:, :w_], cx32[:].bitcast(F32R), xsq[:, c0:c1].bitcast(F32R),
                    start=True, stop=True,
                )
                nc.scalar.copy(mx[:, c0:c1], smx[:, :w_])
                charge("scalar", w_ * 0.8333 + 370)
                nc.gpsimd.tensor_mul(mx2[:, c0:c1], mx[:, c0:c1], mx[:, c0:c1])
                charge("gpsimd", w_ * 0.8333)
                nc.vector.tensor_sub(vx[:, c0:c1], sqx[:, :w_], mx2[:, c0:c1])
                charge("vector", w_ * 1.0417 + 125)
                nc.scalar.activation(sdx[:, c0:c1], vx[:, c0:c1], SQRT, bias=eps32[:])
                charge("scalar", w_ * 0.8333 + 370)
                nc.gpsimd.tensor_sub(xt[:, c0:c1], x_sb[:, c0:c1], mx[:, c0:c1])
                charge("gpsimd", w_ * 0.8333)
            if not simple:
                rsx = cst.tile([C2, N], F32, tag="rsx")
                nc.vector.reciprocal(rsx[:], sdx[:])
                charge("vector", N * 1.0417 + 60)

            # ---- per-chunk q-side + outer product ----
            cn = []
            for md in range(2):
                cnt = cst.tile([128, N], BF16, tag=f"cn{md}")
                cn.append(cnt)
            a_sb = cst.tile([C2, N], BF16, tag="a")
            mb = cst.tile([128, N], F32, tag="mb")

            for ci, (c0, c1) in enumerate(CHUNKS):
                w_ = c1 - c0
                # projection into PSUM
                pj = []
                for md in range(2):
                    p_ = ps.tile([128, 512], F32, tag="ps")
                    for k in range(8):
                        nc.tensor.matmul(
                            p_[:, :w_],
                            w_sb[k][:, 128 * md : 128 * (md + 1)],
                            q_sb[k][:, c0:c1],
                            start=(k == 0),
                            stop=(k == 7),
                        )
                    pj.append(p_)

                # PSUM exit: projb (ACT; +bp in general mode), squares (Pool)
                projb, sq = [], []
                for md in range(2):
                    pb = wrk.tile([128, 512], F32, tag=f"pb{md}")
                    if simple:
                        nc.scalar.copy(pb[:, :w_], pj[md][:, :w_])
                    else:
                        nc.scalar.activation(
                            pb[:, :w_], pj[md][:, :w_], COPY,
                            bias=bp_sb[:, md : md + 1],
                        )
                    charge("scalar", w_ * 0.8333 + 370)
                    projb.append(pb)
                    s = wrk.tile([128, 512], F32, tag=f"sq{md}")
                    nc.gpsimd.tensor_mul(s[:, :w_], pb[:, :w_], pb[:, :w_])
                    charge("gpsimd", w_ * 0.8333)
                    sq.append(s)

                # stats: mean (128 rows) and E[v^2] (32 rows suffice in simple)
                smq = ps.tile([128, 512], F32, tag="ps")
                for md in range(2):
                    nc.tensor.matmul(
                        smq[:, :w_], cq128[:].bitcast(F32R),
                        projb[md][:, :w_].bitcast(F32R),
                        start=(md == 0), stop=(md == 1),
                    )
                qq_rows = C2 if simple else 128
                cqs = cq32 if simple else cq128
                sqq = (ps32 if simple else ps).tile([qq_rows, 512], F32, tag="s32" if simple else "ps")
                for md in range(2):
                    nc.tensor.matmul(
                        sqq[:, :w_], cqs[:].bitcast(F32R),
                        sq[md][:, :w_].bitcast(F32R),
                        start=(md == 0), stop=(md == 1),
                    )

                nc.scalar.copy(mb[:, c0:c1], smq[:, :w_])
                charge("scalar", w_ * 0.8333 + 370)

                mb2 = wrk.tile([qq_rows, 512], F32, tag="mb2")
                nc.gpsimd.tensor_mul(
                    mb2[:, :w_], mb[:qq_rows, c0:c1], mb[:qq_rows, c0:c1]
                )
                charge("gpsimd", w_ * 0.8333)
                varq = wrk.tile([qq_rows, 512], F32, tag="varq")
                nc.vector.tensor_sub(varq[:, :w_], sqq[:, :w_], mb2[:, :w_])
                charge("vector", w_ * 1.0417 + 125)
                sdq = wrk.tile([qq_rows, 512], F32, tag="sdq")
                nc.scalar.activation(sdq[:, :w_], varq[:, :w_], SQRT, bias=eps32[:] if simple else eps128[:])
                charge("scalar", w_ * 0.8333 + 370)
                if not simple:
                    rstd = wrk.tile([128, 512], F32, tag="rstd")
                    nc.vector.reciprocal(rstd[:, :w_], sdq[:, :w_])
                    charge("vector", w_ * 1.0417 + 60)

                # cn tiles (bf16)
                for md in range(2):
                    if simple:
                        nc.gpsimd.tensor_sub(
                            cn[md][:, c0:c1], projb[md][:, :w_], mb[:, c0:c1]
                        )
                        charge("gpsimd", w_ * 0.8333)
                    else:
                        t1 = wrk.tile([128, 512], F32, tag="t1")
                        nc.gpsimd.tensor_sub(t1[:, :w_], projb[md][:, :w_], mb[:, c0:c1])
                        charge("gpsimd", w_ * 0.8333)
                        t2 = wrk.tile([128, 512], F32, tag="t2")
                        nc.gpsimd.tensor_mul(t2[:, :w_], t1[:, :w_], rstd[:, :w_])
                        charge("gpsimd", w_ * 0.8333)
                        nc.vector.tensor_scalar(
                            cn[md][:, c0:c1], t2[:, :w_],
                            g1_sb[:, md : md + 1], b1_sb[:, md : md + 1],
                            op0=MULT, op1=ADD,
                        )
                        charge("vector", w_ * 1.0417 + 60)

                # A' row matrix (bf16): simple folds q-side rstd in via a
                # single fused reciprocal rr = 1/(sd_x * sd_q)
                if simple:
                    pxq = wrk.tile([C2, 512], F32, tag="pxq")
                    nc.gpsimd.tensor_mul(pxq[:, :w_], sdx[:, c0:c1], sdq[:, :w_])
                    charge("gpsimd", w_ * 0.8333)
                    rr = wrk.tile([C2, 512], F32, tag="rr")
                    nc.vector.reciprocal(rr[:, :w_], pxq[:, :w_])
                    charge("vector", w_ * 1.0417 + 60)
                    nc.gpsimd.tensor_mul(a_sb[:, c0:c1], xt[:, c0:c1], rr[:, :w_])
                    charge("gpsimd", w_ * 0.8333)
                else:
                    t3 = wrk.tile([C2, 512], F32, tag="t3")
                    nc.gpsimd.tensor_mul(t3[:, :w_], xt[:, c0:c1], rsx[:, c0:c1])
                    charge("gpsimd", w_ * 0.8333)
                    nc.vector.tensor_scalar(
                        a_sb[:, c0:c1], t3[:, :w_], g2_sb[:, 0:1], b2_sb[:, 0:1],
                        op0=MULT, op1=ADD,
                    )
                    charge("vector", w_ * 1.0417 + 60)

                # A' -> DRAM scratch for broadcast reads
                eng = pick(DMA_ENGS, dma_cost(w_ * 2))
                getattr(nc, eng).dma_start(abuf_d[:, c0:c1], a_sb[:, c0:c1])

                # ---- outer-product tiles ----
                n_pe = PE_ROUTE_E.get(ci, 0)
                xbes = {}

                def make_xbe(e):
                    t = xbp.tile([128, 512], BF16, tag=f"xbe{ci}_{e}")
                    if e < n_pe:
                        xps = ps.tile([128, 512], F32, tag="ps")
                        nc.tensor.matmul(
                            xps[:, :w_], sel_sb[:, 128 * e : 128 * (e + 1)],
                            a_sb[:, c0:c1], start=True, stop=True,
                        )
                        nc.scalar.copy(t[:, :w_], xps[:, :w_])
                        charge("scalar", w_ * 0.8333 + 370)
                    else:
                        src = abuf_d[e : e + 1, c0:c1].partition_broadcast(128)
                        eng = pick(DMA_ENGS, dma_cost(w_ * 2))
                        getattr(nc, eng).dma_start(t[:, :w_], src)
                    xbes[e] = t

                for e in range(min(XBE_PREFETCH, C2)):
                    make_xbe(e)

                for e in range(C2):
                    if e + XBE_PREFETCH < C2:
                        make_xbe(e + XBE_PREFETCH)
                    xbe = xbes.pop(e)
                    st = stg.tile([128, 2 * 512], BF16, tag="st")
                    for md in range(2):
                        eng = pick(["vector", "gpsimd"], mul_cost("vector", w_))
                        # re-charge correct cost if gpsimd won
                        if eng == "gpsimd":
                            charge("gpsimd", mul_cost("gpsimd", w_) - mul_cost("vector", w_))
                        getattr(nc, eng).tensor_mul(
                            st[:, 512 * md : 512 * md + w_],
                            cn[md][:, c0:c1],
                            xbe[:, :w_],
                        )
                    dst = outv[:, :, e, c0:c1]
                    src = st[:].rearrange("p (md n) -> p md n", md=2)[:, :, :w_]
                    eng = pick(DMA_ENGS, dma_cost(2 * w_ * 2))
                    getattr(nc, eng).dma_start(dst, src)

    nc.compile()
    return nc


def _host_inputs(q, x, Wp, bp, g1, b1, g2, b2):
    """Build the 8 per-core input maps."""
    import ml_dtypes

    qf = np.asarray(q, dtype=np.float32).reshape(B, C1, N)
    qfb = qf.astype(ml_dtypes.bfloat16)
    xf = np.ascontiguousarray(np.asarray(x, dtype=np.float32).reshape(B, C2, N))
    wT = np.ascontiguousarray(np.asarray(Wp, dtype=np.float32).T).astype(
        ml_dtypes.bfloat16
    )
    sel = np.zeros((C2, 4 * 128), dtype=ml_dtypes.bfloat16)
    for e in range(4):
        sel[e, 128 * e : 128 * (e + 1)] = 1.0
    bpc = np.ascontiguousarray(np.asarray(bp, dtype=np.float32).reshape(2, 128).T)
    g1c = np.ascontiguousarray(np.asarray(g1, dtype=np.float32).reshape(2, 128).T)
    b1c = np.ascontiguousarray(np.asarray(b1, dtype=np.float32).reshape(2, 128).T)
    g2r = np.ascontiguousarray(np.asarray(g2, dtype=np.float32)[:, None])
    b2r = np.ascontiguousarray(np.asarray(b2, dtype=np.float32)[:, None])
    in_maps = []
    for b in range(B):
        in_maps.append(
            {
                "qb": np.ascontiguousarray(qfb[b]),
                "wT": wT,
                "x": xf[b],
                "selc": sel,
                "bpc": bpc,
                "g1c": g1c,
                "b1c": b1c,
                "g2r": g2r,
                "b2r": b2r,
            }
        )
    return in_maps


def _is_simple(bp, g1, b1, g2, b2):
    return (
        np.allclose(np.asarray(bp), 0)
        and np.allclose(np.asarray(g1), 1)
        and np.allclose(np.asarray(b1), 0)
        and np.allclose(np.asarray(g2), 1)
        and np.allclose(np.asarray(b2), 0)
    )


def _run(in_maps, simple=True, trace=False):
    from concourse.bass_utils import run_bass_kernel_spmd

    key = f"nc{int(simple)}"
    if key not in _CACHE:
        _CACHE[key] = _build_nc(simple)
    nc = _CACHE[key]
    res = run_bass_kernel_spmd(nc, in_maps, core_ids=list(range(B)), trace=trace)
    return res


def kernel(q, x, Wp, bp, g1, b1, g2, b2):
    simple = _is_simple(bp, g1, b1, g2, b2)
    _CACHE["simple"] = simple
    in_maps = _host_inputs(q, x, Wp, bp, g1, b1, g2, b2)
    res = _run(in_maps, simple=simple, trace=False)
    out = np.stack(
        [
            np.asarray(res.results[b]["out"]).astype(np.float32).reshape(CD, H, W)
            for b in range(B)
        ]
    )
    _CACHE["last_res"] = res
    return out


# revision 17
# speedup vs baseline: 2.0578x; 1.0158x over previous
"""Trainium2 Bass kernel for nn_HadaMard: fused proj + 2xLayerNorm + outer product.

Reference computation (per batch b, one NeuronCore per batch):
  qf = q[b].reshape(C1, N)           # [1024, 1024]
  proj = Wp @ qf (+ bp)              # [256, 1024]
  qn = LN_over_d(proj) * g1 + b1     # LN over the 256-channel dim
  xn = LN_over_e(x[b]) * g2 + b2     # LN over the 32-channel dim
  out[d*32+e, n] = qn[d, n] * xn[e, n]   # [8192, 1024]

Layout/strategy:
  - Output is transferred in bf16 (rel-err ~6e-3 << 2e-2 gate) and upcast on
    host: halves the dominant HBM write traffic.
  - Outer-product tiles are e-major: tile (md, e) holds out rows
    (128*md+p)*32+e for p in [0,128). The qn factor is the bf16 qn tile
    itself (no broadcast); the xn factor is one row broadcast to all 128
    partitions.
  - Row broadcasts go through a DRAM scratch roundtrip: A = xn is written
    once (ready early, x-side only), then each xbe tile is a stride-0
    partition-broadcast DMA read. These land on the DMA queues
    (sync/scalar/gpsimd) during the otherwise-idle window while the q-side
    LN chain runs, instead of loading the busy compute engines.
  - Elementwise products run on DVE (bf16 2x mode) and Pool, DMAs on
    sync/scalar/gpsimd, assigned by a static least-loaded balancer.
  - The q side is processed in two 512-column chunks (PSUM bank granularity);
    early e's run per-chunk products to start output DMA sooner, later e's
    run full-width products.
"""

import numpy as np

_CACHE = {}

B, C1, H, W = 8, 1024, 32, 32
C2 = 32
Cp = 256
N = H * W  # 1024
CD = Cp * C2  # 8192
EPS = 1e-5

ESPLIT = 8  # e < ESPLIT: per-chunk products; else full-width
NWU = 10  # PE warm-up matmuls
NFILL = 4  # PE filler matmuls between proj chunks


def _build_nc(simple):
    import concourse.bacc as bacc
    import concourse.bass as bass
    import concourse.mybir as mybir
    import concourse.tile as tile

    F32 = mybir.dt.float32
    F32R = mybir.dt.float32r
    BF16 = mybir.dt.bfloat16
    SQRT = mybir.ActivationFunctionType.Sqrt
    COPY = mybir.ActivationFunctionType.Copy
    MULT = mybir.AluOpType.mult
    ADD = mybir.AluOpType.add

    nc = bacc.Bacc(None, target_bir_lowering=False)

    q_d = nc.dram_tensor("qb", [C1, N], BF16, kind="ExternalInput")
    w_d = nc.dram_tensor("wT", [C1, Cp], BF16, kind="ExternalInput")
    x_d = nc.dram_tensor("x", [C2, N], F32, kind="ExternalInput")
    bp_d = nc.dram_tensor("bpc", [128, 2], F32, kind="ExternalInput")
    g1_d = nc.dram_tensor("g1c", [128, 2], F32, kind="ExternalInput")
    b1_d = nc.dram_tensor("b1c", [128, 2], F32, kind="ExternalInput")
    g2_d = nc.dram_tensor("g2r", [C2, 1], F32, kind="ExternalInput")
    b2_d = nc.dram_tensor("b2r", [C2, 1], F32, kind="ExternalInput")
    abuf_d = nc.dram_tensor("abuf", [C2, N], BF16, kind="Internal")
    out_d = nc.dram_tensor("out", [CD, N], BF16, kind="ExternalOutput")

    # out view: row (md*128+p)*32+e  ->  [p, md, e, n]
    outv = out_d.rearrange("(md p e) n -> p md e n", md=2, p=128, e=C2)

    # ---- static least-loaded balancer (model-cost ns) ----
    clk = {"sync": 0.0, "scalar": 0.0, "gpsimd": 0.0, "vector": 0.0}

    def pick(cands, costs):
        e = min(cands, key=lambda x: clk[x])
        clk[e] += costs[e] if isinstance(costs, dict) else costs
        return e

    def charge(e, cost):
        clk[e] += cost

    DMA_ENGS = ["sync", "scalar", "gpsimd"]

    def dma_cost(bytes_per_part):
        return max(bytes_per_part * 0.3855, 500.0)

    def mul_costs(w):
        return {"vector": w * 1.0417 * 0.5 + 60.0, "gpsimd": w * 0.8333}

    CHUNKS = [(0, 512), (512, 1024)]

    with tile.TileContext(nc) as tc:
        with (
            tc.tile_pool(name="cst", bufs=1) as cst,
            tc.tile_pool(name="big", bufs=1) as big,
            tc.tile_pool(name="xbe", bufs=1) as xbp,
            tc.tile_pool(name="stg", bufs=7) as stg,
            tc.tile_pool(name="stc", bufs=4) as stc,
            tc.tile_pool(name="wrk", bufs=2) as wrk,
            tc.tile_pool(name="ps", bufs=4, space=bass.MemorySpace.PSUM) as ps,
            tc.tile_pool(name="wups", bufs=1, space=bass.MemorySpace.PSUM) as wups,
            tc.tile_pool(name="ps32", bufs=2, space=bass.MemorySpace.PSUM) as ps32,
        ):
            # ---- constants / warmup (t=0, no input deps) ----
            wu_l = cst.tile([128, 128], BF16, tag="wul")
            nc.vector.memset(wu_l[:], 0.5)
            wu_r = cst.tile([128, 512], BF16, tag="wur")
            nc.vector.memset(wu_r[:], 0.5)
            cq128 = cst.tile([128, 128], F32, tag="cq128")
            nc.vector.memset(cq128[:], 1.0 / Cp)
            cx32 = cst.tile([C2, C2], F32, tag="cx32")
            nc.vector.memset(cx32[:], 1.0 / C2)
            eps32 = cst.tile([C2, 1], F32, tag="eps32")
            nc.vector.memset(eps32[:], EPS)
            eps128 = cst.tile([128, 1], F32, tag="eps128")
            nc.vector.memset(eps128[:], EPS)
            # preload the Sqrt activation table early (ACT, off critical path)
            atl = cst.tile([C2, 1], F32, tag="atl")
            nc.scalar.activation(atl[:], eps32[:], SQRT, bias=eps32[:])

            wu_ps = wups.tile([128, 512], F32, tag="wups")
            for i in range(NWU):
                nc.tensor.matmul(wu_ps[:], wu_l[:], wu_r[:], start=True, stop=True)

            def fillers(n):
                for _ in range(n):
                    nc.tensor.matmul(wu_ps[:], wu_l[:], wu_r[:], start=True, stop=True)

            # ---- input loads (balanced across DMA queues) ----
            q_sb = []
            for k in range(8):
                qt = big.tile([128, N], BF16, tag=f"q{k}")
                q_sb.append(qt)
                eng = pick(DMA_ENGS, dma_cost(2048))
                getattr(nc, eng).dma_start(qt[:], q_d[128 * k : 128 * (k + 1), :])
            # w merged: w2[j][p, 256c+d] = wT[128(4j+c)+p, d]
            w_sb = []
            for j in range(2):
                wt = big.tile([128, 4 * Cp], BF16, tag=f"w{j}")
                w_sb.append(wt)
                src = w_d[512 * j : 512 * (j + 1), :].rearrange("(c p) d -> p c d", c=4)
                dst = wt[:].rearrange("p (c d) -> p c d", c=4)
                eng = pick(DMA_ENGS, dma_cost(2048))
                getattr(nc, eng).dma_start(dst, src)

            def wslice(k, md):
                j, c = divmod(k, 4)
                return w_sb[j][:, 256 * c + 128 * md : 256 * c + 128 * (md + 1)]

            x_sb = cst.tile([C2, N], F32, tag="xs")
            eng = pick(DMA_ENGS, dma_cost(4096))
            getattr(nc, eng).dma_start(x_sb[:], x_d[:])

            def cload(dram, shape, tag):
                t = cst.tile(shape, F32, tag=tag)
                eng = pick(DMA_ENGS, dma_cost(shape[1] * 4))
                getattr(nc, eng).dma_start(t[:], dram[:])
                return t

            if not simple:
                bp_sb = cload(bp_d, [128, 2], "bp")
                g1_sb = cload(g1_d, [128, 2], "g1")
                b1_sb = cload(b1_d, [128, 2], "b1")
                g2_sb = cload(g2_d, [C2, 1], "g2")
                b2_sb = cload(b2_d, [C2, 1], "b2")

            # ---- x-side LN (independent of q; A = xn ready early) ----
            xsq = cst.tile([C2, N], F32, tag="xsq")
            nc.gpsimd.tensor_mul(xsq[:], x_sb[:], x_sb[:])
            charge("gpsimd", N * 0.8333)

            mx = cst.tile([C2, N], F32, tag="mx")
            vx = cst.tile([C2, N], F32, tag="vx")
            sqxs = []
            for ci, (c0, c1) in enumerate(CHUNKS):
                w_ = c1 - c0
                smx = ps32.tile([C2, 512], F32, tag="s32")
                nc.tensor.matmul(
                    smx[:, :w_], cx32[:].bitcast(F32R), x_sb[:, c0:c1].bitcast(F32R),
                    start=True, stop=True,
                )
                sqx = ps32.tile([C2, 512], F32, tag="s32")
                nc.tensor.matmul(
                    sqx[:, :w_], cx32[:].bitcast(F32R), xsq[:, c0:c1].bitcast(F32R),
                    start=True, stop=True,
                )
                sqxs.append(sqx)
                nc.scalar.copy(mx[:, c0:c1], smx[:, :w_])
                charge("scalar", w_ * 0.8333 + 370)
            mx2 = cst.tile([C2, N], F32, tag="mx2")
            nc.gpsimd.tensor_mul(mx2[:], mx[:], mx[:])
            charge("gpsimd", N * 0.8333)
            for ci, (c0, c1) in enumerate(CHUNKS):
                w_ = c1 - c0
                nc.vector.tensor_sub(vx[:, c0:c1], sqxs[ci][:, :w_], mx2[:, c0:c1])
                charge("vector", w_ * 1.0417 + 125)
            sdx = cst.tile([C2, N], F32, tag="sdx")
            nc.scalar.activation(sdx[:], vx[:], SQRT, bias=eps32[:])
            charge("scalar", N * 0.8333 + 370)
            rsx = cst.tile([C2, N], F32, tag="rsx")
            nc.vector.reciprocal(rsx[:], sdx[:])
            charge("vector", N * 1.0417 + 60)
            xt = cst.tile([C2, N], F32, tag="xt")
            nc.gpsimd.tensor_sub(xt[:], x_sb[:], mx[:])
            charge("gpsimd", N * 0.8333)
            a_sb = cst.tile([C2, N], BF16, tag="a")
            if simple:
                nc.gpsimd.tensor_mul(a_sb[:], xt[:], rsx[:])
                charge("gpsimd", N * 0.8333)
            else:
                t3 = cst.tile([C2, N], F32, tag="t3")
                nc.gpsimd.tensor_mul(t3[:], xt[:], rsx[:])
                charge("gpsimd", N * 0.8333)
                nc.vector.tensor_scalar(
                    a_sb[:], t3[:], g2_sb[:, 0:1], b2_sb[:, 0:1], op0=MULT, op1=ADD
                )
                charge("vector", N * 1.0417 + 60)

            # A -> DRAM scratch, then all 32 broadcast reads (fills the DMA
            # window while the q-side LN chain runs)
            eng = pick(DMA_ENGS, dma_cost(N * 2))
            getattr(nc, eng).dma_start(abuf_d[:], a_sb[:])
            xbes = []
            for e in range(C2):
                t = xbp.tile([128, N], BF16, tag=f"xbe{e}")
                src = abuf_d[e : e + 1, :].partition_broadcast(128)
                eng = pick(DMA_ENGS, dma_cost(N * 2))
                getattr(nc, eng).dma_start(t[:], src)
                xbes.append(t)

            # ---- q-side: proj + LN -> cn (bf16, rstd folded in) ----
            cn = []
            for md in range(2):
                cnt = cst.tile([128, N], BF16, tag=f"cn{md}")
                cn.append(cnt)
            mb = cst.tile([128, N], F32, tag="mb")

            def qside_chunk(ci):
                c0, c1 = CHUNKS[ci]
                w_ = c1 - c0
                pj = []
                for md in range(2):
                    p_ = ps.tile([128, 512], F32, tag="ps")
                    for k in range(8):
                        nc.tensor.matmul(
                            p_[:, :w_], wslice(k, md), q_sb[k][:, c0:c1],
                            start=(k == 0), stop=(k == 7),
                        )
                    pj.append(p_)
                fillers(NFILL)

                projb, sq = [], []
                for md in range(2):
                    pb = wrk.tile([128, 512], F32, tag=f"pb{md}")
                    if simple:
                        nc.scalar.copy(pb[:, :w_], pj[md][:, :w_])
                    else:
                        nc.scalar.activation(
                            pb[:, :w_], pj[md][:, :w_], COPY,
                            bias=bp_sb[:, md : md + 1],
                        )
                    charge("scalar", w_ * 0.8333 + 370)
                    projb.append(pb)
                    s = wrk.tile([128, 512], F32, tag=f"sq{md}")
                    nc.gpsimd.tensor_mul(s[:, :w_], pb[:, :w_], pb[:, :w_])
                    charge("gpsimd", w_ * 0.8333)
                    sq.append(s)

                smq = ps.tile([128, 512], F32, tag="ps")
                for md in range(2):
                    nc.tensor.matmul(
                        smq[:, :w_], cq128[:].bitcast(F32R),
                        projb[md][:, :w_].bitcast(F32R),
                        start=(md == 0), stop=(md == 1),
                    )
                sqq = ps.tile([128, 512], F32, tag="ps")
                for md in range(2):
                    nc.tensor.matmul(
                        sqq[:, :w_], cq128[:].bitcast(F32R),
                        sq[md][:, :w_].bitcast(F32R),
                        start=(md == 0), stop=(md == 1),
                    )
                fillers(NFILL)

                nc.scalar.copy(mb[:, c0:c1], smq[:, :w_])
                charge("scalar", w_ * 0.8333 + 370)
                mb2 = wrk.tile([128, 512], F32, tag="mb2")
                nc.gpsimd.tensor_mul(mb2[:, :w_], mb[:, c0:c1], mb[:, c0:c1])
                charge("gpsimd", w_ * 0.8333)
                varq = wrk.tile([128, 512], F32, tag="varq")
                nc.vector.tensor_sub(varq[:, :w_], sqq[:, :w_], mb2[:, :w_])
                charge("vector", w_ * 1.0417 + 125)
                sdq = wrk.tile([128, 512], F32, tag="sdq")
                nc.scalar.activation(sdq[:, :w_], varq[:, :w_], SQRT, bias=eps128[:])
                charge("scalar", w_ * 0.8333 + 370)
                rstd = wrk.tile([128, 512], F32, tag="rstd")
                nc.vector.reciprocal(rstd[:, :w_], sdq[:, :w_])
                charge("vector", w_ * 1.0417 + 60)

                for md in range(2):
                    cs = wrk.tile([128, 512], F32, tag=f"cs{md}")
                    nc.gpsimd.tensor_sub(cs[:, :w_], projb[md][:, :w_], mb[:, c0:c1])
                    charge("gpsimd", w_ * 0.8333)
                    if simple:
                        nc.gpsimd.tensor_mul(cn[md][:, c0:c1], cs[:, :w_], rstd[:, :w_])
                        charge("gpsimd", w_ * 0.8333)
                    else:
                        c2_ = wrk.tile([128, 512], F32, tag=f"c2_{md}")
                        nc.gpsimd.tensor_mul(c2_[:, :w_], cs[:, :w_], rstd[:, :w_])
                        charge("gpsimd", w_ * 0.8333)
                        nc.vector.tensor_scalar(
                            cn[md][:, c0:c1], c2_[:, :w_],
                            g1_sb[:, md : md + 1], b1_sb[:, md : md + 1],
                            op0=MULT, op1=ADD,
                        )
                        charge("vector", w_ * 1.0417 + 60)

            def emit_tile(e, c0, c1):
                """products + staging + out DMA for tile column range [c0,c1)."""
                w_ = c1 - c0
                if w_ == N:
                    st = stg.tile([128, 2 * N], BF16, tag="st")
                else:
                    st = stc.tile([128, 2 * 512], BF16, tag="stc")
                sw = st.shape[1] // 2
                for md in range(2):
                    eng = pick(["vector", "gpsimd"], mul_costs(w_))
                    getattr(nc, eng).tensor_mul(
                        st[:, sw * md : sw * md + w_],
                        cn[md][:, c0:c1],
                        xbes[e][:, c0:c1],
                    )
                dst = outv[:, :, e, c0:c1]
                src = st[:].rearrange("p (md n) -> p md n", md=2)[:, :, :w_]
                eng = pick(DMA_ENGS, dma_cost(2 * w_ * 2))
                getattr(nc, eng).dma_start(dst, src)

            qside_chunk(0)
            for e in range(ESPLIT):
                emit_tile(e, 0, 512)
            qside_chunk(1)
            for e in range(ESPLIT, C2):
                emit_tile(e, 0, N)
            for e in range(ESPLIT):
                emit_tile(e, 512, N)

    nc.compile()
    return nc


def _host_inputs(q, x, Wp, bp, g1, b1, g2, b2):
    """Build the 8 per-core input maps."""
    import ml_dtypes

    qf = np.asarray(q, dtype=np.float32).reshape(B, C1, N)
    qfb = qf.astype(ml_dtypes.bfloat16)
    xf = np.ascontiguousarray(np.asarray(x, dtype=np.float32).reshape(B, C2, N))
    wT = np.ascontiguousarray(np.asarray(Wp, dtype=np.float32).T).astype(
        ml_dtypes.bfloat16
    )
    bpc = np.ascontiguousarray(np.asarray(bp, dtype=np.float32).reshape(2, 128).T)
    g1c = np.ascontiguousarray(np.asarray(g1, dtype=np.float32).reshape(2, 128).T)
    b1c = np.ascontiguousarray(np.asarray(b1, dtype=np.float32).reshape(2, 128).T)
    g2r = np.ascontiguousarray(np.asarray(g2, dtype=np.float32)[:, None])
    b2r = np.ascontiguousarray(np.asarray(b2, dtype=np.float32)[:, None])
    in_maps = []
    for b in range(B):
        in_maps.append(
            {
                "qb": np.ascontiguousarray(qfb[b]),
                "wT": wT,
                "x": xf[b],
                "bpc": bpc,
                "g1c": g1c,
                "b1c": b1c,
                "g2r": g2r,
                "b2r": b2r,
            }
        )
    return in_maps


def _is_simple(bp, g1, b1, g2, b2):
    return (
        np.allclose(np.asarray(bp), 0)
        and np.allclose(np.asarray(g1), 1)
        and np.allclose(np.asarray(b1), 0)
        and np.allclose(np.asarray(g2), 1)
        and np.allclose(np.asarray(b2), 0)
    )


def _run(in_maps, simple=True, trace=False):
    from concourse.bass_utils import run_bass_kernel_spmd

    key = f"nc{int(simple)}"
    if key not in _CACHE:
        _CACHE[key] = _build_nc(simple)
    nc = _CACHE[key]
    res = run_bass_kernel_spmd(nc, in_maps, core_ids=list(range(B)), trace=trace)
    return res


def kernel(q, x, Wp, bp, g1, b1, g2, b2):
    simple = _is_simple(bp, g1, b1, g2, b2)
    _CACHE["simple"] = simple
    in_maps = _host_inputs(q, x, Wp, bp, g1, b1, g2, b2)
    res = _run(in_maps, simple=simple, trace=False)
    out = np.stack(
        [
            np.asarray(res.results[b]["out"]).astype(np.float32).reshape(CD, H, W)
            for b in range(B)
        ]
    )
    _CACHE["last_res"] = res
    return out


# revision 18
# speedup vs baseline: 2.2274x; 1.0824x over previous
"""Trainium2 Bass kernel for nn_HadaMard: fused proj + 2xLayerNorm + outer product.

Reference computation (per batch b, one NeuronCore per batch):
  qf = q[b].reshape(C1, N)           # [1024, 1024]
  proj = Wp @ qf (+ bp)              # [256, 1024]
  qn = LN_over_d(proj) * g1 + b1     # LN over the 256-channel dim
  xn = LN_over_e(x[b]) * g2 + b2     # LN over the 32-channel dim
  out[d*32+e, n] = qn[d, n] * xn[e, n]   # [8192, 1024]

Layout/strategy:
  - Output is transferred in bf16 (rel-err ~6e-3 << 2e-2 gate) and upcast on
    host: halves the dominant HBM write traffic.
  - Outer-product tiles are e-major: tile (md, e) holds out rows
    (128*md+p)*32+e for p in [0,128). The qn factor is the bf16 qn tile
    itself (no broadcast); the xn factor is one row broadcast to all 128
    partitions.
  - Row broadcasts go through a DRAM scratch roundtrip: A = xn is written
    once (ready early, x-side only), then each xbe tile is a stride-0
    partition-broadcast DMA read. These land on the DMA queues
    (sync/scalar/gpsimd) during the otherwise-idle window while the q-side
    LN chain runs, instead of loading the busy compute engines.
  - Elementwise products run on DVE (bf16 2x mode) and Pool, DMAs on
    sync/scalar/gpsimd, assigned by a static least-loaded balancer.
  - The q side is processed in two 512-column chunks (PSUM bank granularity);
    early e's run per-chunk products to start output DMA sooner, later e's
    run full-width products.
"""

import numpy as np

_CACHE = {}

B, C1, H, W = 8, 1024, 32, 32
C2 = 32
Cp = 256
N = H * W  # 1024
CD = Cp * C2  # 8192
EPS = 1e-5

ESPLIT = 8  # e < ESPLIT: per-chunk products; else full-width
NWU = 6  # PE warm-up matmuls
NFILL = 4  # PE filler matmuls between proj chunks


def _build_nc(simple):
    import concourse.bacc as bacc
    import concourse.bass as bass
    import concourse.mybir as mybir
    import concourse.tile as tile

    F32 = mybir.dt.float32
    F32R = mybir.dt.float32r
    BF16 = mybir.dt.bfloat16
    SQRT = mybir.ActivationFunctionType.Sqrt
    COPY = mybir.ActivationFunctionType.Copy
    MULT = mybir.AluOpType.mult
    ADD = mybir.AluOpType.add

    nc = bacc.Bacc(None, target_bir_lowering=False)

    q_d = nc.dram_tensor("qb", [C1, N], BF16, kind="ExternalInput")
    w_d = nc.dram_tensor("wT", [C1, Cp], BF16, kind="ExternalInput")
    x_d = nc.dram_tensor("x", [C2, N], F32, kind="ExternalInput")
    bp_d = nc.dram_tensor("bpc", [128, 2], F32, kind="ExternalInput")
    g1_d = nc.dram_tensor("g1c", [128, 2], F32, kind="ExternalInput")
    b1_d = nc.dram_tensor("b1c", [128, 2], F32, kind="ExternalInput")
    g2_d = nc.dram_tensor("g2r", [C2, 1], F32, kind="ExternalInput")
    b2_d = nc.dram_tensor("b2r", [C2, 1], F32, kind="ExternalInput")
    abuf_d = nc.dram_tensor("abuf", [C2, N], BF16, kind="Internal")
    out_d = nc.dram_tensor("out", [CD, N], BF16, kind="ExternalOutput")

    # out view: row (md*128+p)*32+e  ->  [p, md, e, n]
    outv = out_d.rearrange("(md p e) n -> p md e n", md=2, p=128, e=C2)

    # ---- static least-loaded balancer (model-cost ns) ----
    clk = {"sync": 0.0, "scalar": 0.0, "gpsimd": 0.0, "vector": 0.0}

    def pick(cands, costs):
        e = min(cands, key=lambda x: clk[x])
        clk[e] += costs[e] if isinstance(costs, dict) else costs
        return e

    def charge(e, cost):
        clk[e] += cost

    DMA_ENGS = ["sync", "scalar", "gpsimd"]

    def dma_cost(bytes_per_part):
        return max(bytes_per_part * 0.3855, 500.0)

    def mul_costs(w):
        return {"vector": w * 1.0417 * 0.5 + 60.0, "gpsimd": w * 0.8333}

    CHUNKS = [(0, 512), (512, 1024)]

    with tile.TileContext(nc) as tc:
        with (
            tc.tile_pool(name="cst", bufs=1) as cst,
            tc.tile_pool(name="big", bufs=1) as big,
            tc.tile_pool(name="xbe", bufs=1) as xbp,
            tc.tile_pool(name="stg", bufs=7) as stg,
            tc.tile_pool(name="stc", bufs=4) as stc,
            tc.tile_pool(name="wrk", bufs=2) as wrk,
            tc.tile_pool(name="ps", bufs=4, space=bass.MemorySpace.PSUM) as ps,
            tc.tile_pool(name="wups", bufs=1, space=bass.MemorySpace.PSUM) as wups,
            tc.tile_pool(name="ps32", bufs=2, space=bass.MemorySpace.PSUM) as ps32,
        ):
            # ---- constants / warmup (t=0, no input deps) ----
            wu_l = cst.tile([128, 128], BF16, tag="wul")
            nc.vector.memset(wu_l[:], 0.5)
            wu_r = cst.tile([128, 512], BF16, tag="wur")
            nc.vector.memset(wu_r[:], 0.5)
            cq128 = cst.tile([128, 128], F32, tag="cq128")
            nc.vector.memset(cq128[:], 1.0 / Cp)
            cx32 = cst.tile([C2, C2], F32, tag="cx32")
            nc.vector.memset(cx32[:], 1.0 / C2)
            eps32 = cst.tile([C2, 1], F32, tag="eps32")
            nc.vector.memset(eps32[:], EPS)
            eps128 = cst.tile([128, 1], F32, tag="eps128")
            nc.vector.memset(eps128[:], EPS)
            # preload the Sqrt activation table early (ACT, off critical path)
            atl = cst.tile([C2, 1], F32, tag="atl")
            nc.scalar.activation(atl[:], eps32[:], SQRT, bias=eps32[:])

            wu_ps = wups.tile([128, 512], F32, tag="wups")
            for i in range(NWU):
                nc.tensor.matmul(wu_ps[:], wu_l[:], wu_r[:], start=True, stop=True)

            def fillers(n):
                for _ in range(n):
                    nc.tensor.matmul(wu_ps[:], wu_l[:], wu_r[:], start=True, stop=True)

            # ---- input loads: x first (x-side chain is latency-critical) ----
            x_sb = cst.tile([C2, N], F32, tag="xs")
            eng = pick(DMA_ENGS, dma_cost(4096))
            getattr(nc, eng).dma_start(x_sb[:], x_d[:])
            q_sb = []
            for k in range(8):
                qt = big.tile([128, N], BF16, tag=f"q{k}")
                q_sb.append(qt)
                eng = pick(DMA_ENGS, dma_cost(2048))
                getattr(nc, eng).dma_start(qt[:], q_d[128 * k : 128 * (k + 1), :])
            # w merged: w2[j][p, 256c+d] = wT[128(4j+c)+p, d]
            w_sb = []
            for j in range(2):
                wt = big.tile([128, 4 * Cp], BF16, tag=f"w{j}")
                w_sb.append(wt)
                src = w_d[512 * j : 512 * (j + 1), :].rearrange("(c p) d -> p c d", c=4)
                dst = wt[:].rearrange("p (c d) -> p c d", c=4)
                eng = pick(DMA_ENGS, dma_cost(2048))
                getattr(nc, eng).dma_start(dst, src)

            def wslice(k, md):
                j, c = divmod(k, 4)
                return w_sb[j][:, 256 * c + 128 * md : 256 * c + 128 * (md + 1)]

            def cload(dram, shape, tag):
                t = cst.tile(shape, F32, tag=tag)
                eng = pick(DMA_ENGS, dma_cost(shape[1] * 4))
                getattr(nc, eng).dma_start(t[:], dram[:])
                return t

            if not simple:
                bp_sb = cload(bp_d, [128, 2], "bp")
                g1_sb = cload(g1_d, [128, 2], "g1")
                b1_sb = cload(b1_d, [128, 2], "b1")
                g2_sb = cload(g2_d, [C2, 1], "g2")
                b2_sb = cload(b2_d, [C2, 1], "b2")

            # ---- x-side LN (independent of q; A = xn ready early) ----
            xsq = cst.tile([C2, N], F32, tag="xsq")
            nc.gpsimd.tensor_mul(xsq[:], x_sb[:], x_sb[:])
            charge("gpsimd", N * 0.8333)

            mx = cst.tile([C2, N], F32, tag="mx")
            vx = cst.tile([C2, N], F32, tag="vx")
            sqxs = []
            for ci, (c0, c1) in enumerate(CHUNKS):
                w_ = c1 - c0
                smx = ps32.tile([C2, 512], F32, tag="s32")
                nc.tensor.matmul(
                    smx[:, :w_], cx32[:].bitcast(F32R), x_sb[:, c0:c1].bitcast(F32R),
                    start=True, stop=True,
                )
                sqx = ps32.tile([C2, 512], F32, tag="s32")
                nc.tensor.matmul(
                    sqx[:, :w_], cx32[:].bitcast(F32R), xsq[:, c0:c1].bitcast(F32R),
                    start=True, stop=True,
                )
                sqxs.append(sqx)
                nc.scalar.copy(mx[:, c0:c1], smx[:, :w_])
                charge("scalar", w_ * 0.8333 + 370)
            mx2 = cst.tile([C2, N], F32, tag="mx2")
            nc.gpsimd.tensor_mul(mx2[:], mx[:], mx[:])
            charge("gpsimd", N * 0.8333)
            for ci, (c0, c1) in enumerate(CHUNKS):
                w_ = c1 - c0
                nc.vector.tensor_sub(vx[:, c0:c1], sqxs[ci][:, :w_], mx2[:, c0:c1])
                charge("vector", w_ * 1.0417 + 125)
            sdx = cst.tile([C2, N], F32, tag="sdx")
            nc.scalar.activation(sdx[:], vx[:], SQRT, bias=eps32[:])
            charge("scalar", N * 0.8333 + 370)
            rsx = cst.tile([C2, N], F32, tag="rsx")
            nc.vector.reciprocal(rsx[:], sdx[:])
            charge("vector", N * 1.0417 + 60)
            xt = cst.tile([C2, N], F32, tag="xt")
            nc.gpsimd.tensor_sub(xt[:], x_sb[:], mx[:])
            charge("gpsimd", N * 0.8333)
            a_sb = cst.tile([C2, N], BF16, tag="a")
            if simple:
                nc.gpsimd.tensor_mul(a_sb[:], xt[:], rsx[:])
                charge("gpsimd", N * 0.8333)
            else:
                t3 = cst.tile([C2, N], F32, tag="t3")
                nc.gpsimd.tensor_mul(t3[:], xt[:], rsx[:])
                charge("gpsimd", N * 0.8333)
                nc.vector.tensor_scalar(
                    a_sb[:], t3[:], g2_sb[:, 0:1], b2_sb[:, 0:1], op0=MULT, op1=ADD
                )
                charge("vector", N * 1.0417 + 60)

            # A -> DRAM scratch, then all 32 broadcast reads (fills the DMA
            # window while the q-side LN chain runs)
            eng = pick(DMA_ENGS, dma_cost(N * 2))
            getattr(nc, eng).dma_start(abuf_d[:], a_sb[:])
            xbes = []
            for e in range(C2):
                t = xbp.tile([128, N], BF16, tag=f"xbe{e}")
                src = abuf_d[e : e + 1, :].partition_broadcast(128)
                eng = DMA_ENGS[e % 3]
                charge(eng, dma_cost(N * 2))
                getattr(nc, eng).dma_start(t[:], src)
                xbes.append(t)

            # ---- q-side: proj + LN -> cn (bf16, rstd folded in) ----
            cn = []
            for md in range(2):
                cnt = cst.tile([128, N], BF16, tag=f"cn{md}")
                cn.append(cnt)
            mb = cst.tile([128, N], F32, tag="mb")

            def qside_chunk(ci):
                c0, c1 = CHUNKS[ci]
                w_ = c1 - c0
                pj = []
                for md in range(2):
                    p_ = ps.tile([128, 512], F32, tag="ps")
                    for k in range(8):
                        nc.tensor.matmul(
                            p_[:, :w_], wslice(k, md), q_sb[k][:, c0:c1],
                            start=(k == 0), stop=(k == 7),
                        )
                    pj.append(p_)
                fillers(NFILL)

                projb, sq = [], []
                for md in range(2):
                    pb = wrk.tile([128, 512], F32, tag=f"pb{md}")
                    if simple:
                        nc.scalar.copy(pb[:, :w_], pj[md][:, :w_])
                    else:
                        nc.scalar.activation(
                            pb[:, :w_], pj[md][:, :w_], COPY,
                            bias=bp_sb[:, md : md + 1],
                        )
                    charge("scalar", w_ * 0.8333 + 370)
                    projb.append(pb)
                    s = wrk.tile([128, 512], F32, tag=f"sq{md}")
                    nc.gpsimd.tensor_mul(s[:, :w_], pb[:, :w_], pb[:, :w_])
                    charge("gpsimd", w_ * 0.8333)
                    sq.append(s)

                smq = ps.tile([128, 512], F32, tag="ps")
                for md in range(2):
                    nc.tensor.matmul(
                        smq[:, :w_], cq128[:].bitcast(F32R),
                        projb[md][:, :w_].bitcast(F32R),
                        start=(md == 0), stop=(md == 1),
                    )
                sqq = ps.tile([128, 512], F32, tag="ps")
                for md in range(2):
                    nc.tensor.matmul(
                        sqq[:, :w_], cq128[:].bitcast(F32R),
                        sq[md][:, :w_].bitcast(F32R),
                        start=(md == 0), stop=(md == 1),
                    )
                fillers(NFILL)

                nc.scalar.copy(mb[:, c0:c1], smq[:, :w_])
                charge("scalar", w_ * 0.8333 + 370)
                mb2 = wrk.tile([128, 512], F32, tag="mb2")
                nc.gpsimd.tensor_mul(mb2[:, :w_], mb[:, c0:c1], mb[:, c0:c1])
                charge("gpsimd", w_ * 0.8333)
                varq = wrk.tile([128, 512], F32, tag="varq")
                nc.vector.tensor_sub(varq[:, :w_], sqq[:, :w_], mb2[:, :w_])
                charge("vector", w_ * 1.0417 + 125)
                sdq = wrk.tile([128, 512], F32, tag="sdq")
                nc.scalar.activation(sdq[:, :w_], varq[:, :w_], SQRT, bias=eps128[:])
                charge("scalar", w_ * 0.8333 + 370)
                rstd = wrk.tile([128, 512], F32, tag="rstd")
                nc.vector.reciprocal(rstd[:, :w_], sdq[:, :w_])
                charge("vector", w_ * 1.0417 + 60)

                for md in range(2):
                    cs = wrk.tile([128, 512], F32, tag=f"cs{md}")
                    nc.gpsimd.tensor_sub(cs[:, :w_], projb[md][:, :w_], mb[:, c0:c1])
                    charge("gpsimd", w_ * 0.8333)
                    if simple:
                        nc.gpsimd.tensor_mul(cn[md][:, c0:c1], cs[:, :w_], rstd[:, :w_])
                        charge("gpsimd", w_ * 0.8333)
                    else:
                        c2_ = wrk.tile([128, 512], F32, tag=f"c2_{md}")
                        nc.gpsimd.tensor_mul(c2_[:, :w_], cs[:, :w_], rstd[:, :w_])
                        charge("gpsimd", w_ * 0.8333)
                        nc.vector.tensor_scalar(
                            cn[md][:, c0:c1], c2_[:, :w_],
                            g1_sb[:, md : md + 1], b1_sb[:, md : md + 1],
                            op0=MULT, op1=ADD,
                        )
                        charge("vector", w_ * 1.0417 + 60)

            POOL_E = {2, 5, 7}  # e % 8 in POOL_E -> Pool owns both products
            out_rr = [0]

            def emit_tile(e, c0, c1):
                """products + staging + out DMA for tile column range [c0,c1)."""
                w_ = c1 - c0
                if w_ == N:
                    st = stg.tile([128, 2 * N], BF16, tag="st")
                else:
                    st = stc.tile([128, 2 * 512], BF16, tag="stc")
                sw = st.shape[1] // 2
                meng = "gpsimd" if (e % 8) in POOL_E else "vector"
                for md in range(2):
                    charge(meng, mul_costs(w_)[meng])
                    getattr(nc, meng).tensor_mul(
                        st[:, sw * md : sw * md + w_],
                        cn[md][:, c0:c1],
                        xbes[e][:, c0:c1],
                    )
                dst = outv[:, :, e, c0:c1]
                src = st[:].rearrange("p (md n) -> p md n", md=2)[:, :, :w_]
                eng = ["sync", "scalar"][out_rr[0] % 2]
                out_rr[0] += 1
                charge(eng, dma_cost(2 * w_ * 2))
                getattr(nc, eng).dma_start(dst, src)

            qside_chunk(0)
            for e in range(ESPLIT):
                emit_tile(e, 0, 512)
            qside_chunk(1)
            for e in range(ESPLIT, C2):
                emit_tile(e, 0, N)
            for e in range(ESPLIT):
                emit_tile(e, 512, N)

    nc.compile()
    return nc


def _host_inputs(q, x, Wp, bp, g1, b1, g2, b2):
    """Build the 8 per-core input maps."""
    import ml_dtypes

    qf = np.asarray(q, dtype=np.float32).reshape(B, C1, N)
    qfb = qf.astype(ml_dtypes.bfloat16)
    xf = np.ascontiguousarray(np.asarray(x, dtype=np.float32).reshape(B, C2, N))
    wT = np.ascontiguousarray(np.asarray(Wp, dtype=np.float32).T).astype(
        ml_dtypes.bfloat16
    )
    bpc = np.ascontiguousarray(np.asarray(bp, dtype=np.float32).reshape(2, 128).T)
    g1c = np.ascontiguousarray(np.asarray(g1, dtype=np.float32).reshape(2, 128).T)
    b1c = np.ascontiguousarray(np.asarray(b1, dtype=np.float32).reshape(2, 128).T)
    g2r = np.ascontiguousarray(np.asarray(g2, dtype=np.float32)[:, None])
    b2r = np.ascontiguousarray(np.asarray(b2, dtype=np.float32)[:, None])
    in_maps = []
    for b in range(B):
        in_maps.append(
            {
                "qb": np.ascontiguousarray(qfb[b]),
                "wT": wT,
                "x": xf[b],
                "bpc": bpc,
                "g1c": g1c,
                "b1c": b1c,
                "g2r": g2r,
                "b2r": b2r,
            }
        )
    return in_maps


def _is_simple(bp, g1, b1, g2, b2):
    return (
        np.allclose(np.asarray(bp), 0)
        and np.allclose(np.asarray(g1), 1)
        and np.allclose(np.asarray(b1), 0)
        and np.allclose(np.asarray(g2), 1)
        and np.allclose(np.asarray(b2), 0)
    )


def _run(in_maps, simple=True, trace=False):
    from concourse.bass_utils import run_bass_kernel_spmd

    key = f"nc{int(simple)}"
    if key not in _CACHE:
        _CACHE[key] = _build_nc(simple)
    nc = _CACHE[key]
    res = run_bass_kernel_spmd(nc, in_maps, core_ids=list(range(B)), trace=trace)
    return res


def kernel(q, x, Wp, bp, g1, b1, g2, b2):
    simple = _is_simple(bp, g1, b1, g2, b2)
    _CACHE["simple"] = simple
    in_maps = _host_inputs(q, x, Wp, bp, g1, b1, g2, b2)
    res = _run(in_maps, simple=simple, trace=False)
    out = np.stack(
        [
            np.asarray(res.results[b]["out"]).astype(np.float32).reshape(CD, H, W)
            for b in range(B)
        ]
    )
    _CACHE["last_res"] = res
    return out
